# revision 21
# baseline (speedup 1.0000x reference)
"""Trainium2 Bass kernel for a 2-layer GRU teacher-forced decoder.

Math (per reference):
  toks[t,b]: t=0 -> SOS(=1), t>=1 -> target[b, t]   (T = ML-1 = 63 steps)
  x_t = relu(emb[toks[t]])                          [B, E]
  h0 <- GRUCell(x_t, h0; W_ih0, W_hh0, b_ih0, b_hh0)
  h1 <- GRUCell(h0, h1; W_ih1, W_hh1, b_ih1, b_hh1)
  logits_t = h1 @ W_out.T + b_out                   [B, V]
  out = stack(logits).transpose(1,0,2)              [B, T, V]

Device strategy (8 cores, SPMD, no collectives):
  - The sequential GRU recurrence is replicated on every core; the large
    output projection is sharded column-wise (vocab) 8 ways.

Performance structure (v4):
  - The L0 input path gi0 = relu(emb[tok]) @ W_ih0.T (+ input-side
    biases) depends only on the teacher-forced tokens, so it is computed
    on the host and streamed per-step into SBUF.
  - gi0's r,z part and L1's r,z biases are accumulated straight into
    PSUM by an identity-stationary pass-through matmul, so the sigmoid
    reads PSUM directly and the wide DVE adds disappear.
  - Gate elementwise is all-bf16 and split across engines: DVE does the
    multiplies/adds on the critical path, the scalar engine does
    sigmoid (r-half first) and tanh, gpsimd does the n-path PSUM adds,
    fp32->bf16 state copies, and the h1 history mirror.
  - PE group order per iteration: G1rec(t-1), G1inp(t-1)+bias, G0(t)
    +gi0 — chosen so each layer's elementwise chain lands just in time
    for the matmul group that consumes it in the next iteration.
  - h1 states transpose into a ring and are mirrored into an
    SBUF-resident history buffer [128, k, 32t+b]; the vocab-sharded
    logits GEMM at the end reads its stationary m-tiles straight from
    this buffer (no HBM round trip).

Layouts:
  - Quarter layout for gates/state: [32q+b, j] = value(batch b, hidden
    q*256+j).
  - Permuted-hidden layout for tensors contracted over H (_perm_rows):
    matches what the DVE 32x32 block transpose produces.
  - The output-projection bias is added on the host after gathering the
    vocab shards.
"""

import os
import sys
import numpy as np

sys.path.insert(0, "/opt/trn_rl_repo")

import ml_dtypes

V, E, H, B, ML = 32000, 512, 1024, 32, 64
SOS = 1
T = ML - 1          # 63
TB = T * B          # 2016
NCORES = 8
VS = V // NCORES    # 4000 vocab slice per core
Q = 4               # hidden quarters
J = H // Q          # 256
KH = H // 128       # 8 contraction chunks over H
MT = 126            # logits M-tile (2016 = 16 * 126)
NMT = TB // MT      # 16
NS = 500            # logits psum slice width (one 2KB fp32 bank)

_BF = ml_dtypes.bfloat16


def _bf16(x):
    return np.asarray(x, np.float32).astype(_BF)


def _prep_hq(h):
    """h: [B, H] fp32 -> quarter layout [128, 256] bf16."""
    hq = np.asarray(h, np.float32).reshape(B, Q, J).transpose(1, 0, 2).reshape(Q * B, J)
    return np.ascontiguousarray(hq)


# Permuted hidden layout used for every tensor whose contraction is over H.
# Position (p=32q+i, c) holds hidden unit 256q + 32c + i, which is exactly
# what the DVE 32x32 block transpose produces from the quarter-layout state
# [32q+b, 256q-relative j=32c+i]: out[32q+i, (c,b)] = h[b, 256q+32c+i].

def _perm_rows(m):
    """m: [H, cols] -> [128, KH, cols] in the permuted-hidden layout."""
    cols = m.shape[1]
    return np.ascontiguousarray(
        m.reshape(Q, KH, 32, cols).transpose(0, 2, 1, 3).reshape(128, KH, cols))


def _prep_wT_perm(w):
    """w: [3H, H] fp32 -> [128, KH, 3H] bf16, permuted-hidden contraction."""
    wt = np.ascontiguousarray(np.asarray(w, np.float32).T)       # [H, 3H]
    return _bf16(_perm_rows(wt))


def _prep_hT_perm(h):
    """h: [B, H] -> [128, KH, B] bf16, permuted-hidden layout."""
    ht = np.ascontiguousarray(np.asarray(h, np.float32).T)       # [H, B]
    return _bf16(_perm_rows(ht))


def _gate_bias_quarter(b_ih, b_hh):
    bi = np.asarray(b_ih, np.float32)
    bh = np.asarray(b_hh, np.float32)
    comb = np.empty(3 * H, np.float32)
    comb[0:2 * H] = bi[0:2 * H] + bh[0:2 * H]
    comb[2 * H:] = bh[2 * H:]
    bq = np.empty((128, 3 * J), np.float32)
    bc = np.empty((128, J), np.float32)
    for q in range(Q):
        s = q * J
        row = np.concatenate([comb[s:s + J], comb[H + s:H + s + J],
                              comb[2 * H + s:2 * H + s + J]])
        bq[32 * q:32 * (q + 1)] = row[None, :]
        bc[32 * q:32 * (q + 1)] = bi[2 * H + s:2 * H + s + J][None, :]
    return bq, bc


def _quarter_rows(vec, width):
    """[4*width] fp32 -> [128, width]: row 32q+b = vec[q*width:(q+1)*width]."""
    out = np.empty((128, width), np.float32)
    v = np.asarray(vec, np.float32)
    for q in range(Q):
        out[32 * q:32 * (q + 1)] = v[q * width:(q + 1) * width][None, :]
    return out


def _host_gi0(target_tensor, emb, W_ih0, b_ih0, b_hh0):
    """Precompute the L0 input path on the host.

    Returns [128, T, 3J] bf16 quarter-layout:
      cols [0:2J)  = x@Wih0.T (r,z) + (b_ih0+b_hh0) r,z parts
      cols [2J:3J) = x@Wih0.T (n)   + b_ih0 n part
    """
    tt = np.asarray(target_tensor)
    toks = np.concatenate(
        [np.full((B, 1), SOS, dtype=tt.dtype), tt[:, 1:ML - 1]], axis=1).T  # [T, B]
    X = np.maximum(np.asarray(emb, np.float32)[toks], 0.0)       # [T, B, E]
    Gi = X.reshape(TB, E) @ np.asarray(W_ih0, np.float32).T      # [TB, 3H]
    bias = np.empty(3 * H, np.float32)
    bias[0:2 * H] = np.asarray(b_ih0, np.float32)[0:2 * H] + \
        np.asarray(b_hh0, np.float32)[0:2 * H]
    bias[2 * H:] = np.asarray(b_ih0, np.float32)[2 * H:]
    Gi += bias[None, :]
    G5 = Gi.reshape(T, B, 3, Q, J)                  # col = g*H + q*J + j
    gq = G5.transpose(3, 1, 0, 2, 4).reshape(Q * B, T, 3 * J)
    return np.ascontiguousarray(_bf16(gq))


def _build_inputs(encoder_hidden, target_tensor, emb,
                  W_ih0, W_hh0, b_ih0, b_hh0, W_ih1, W_hh1, b_ih1, b_hh1,
                  W_out, b_out):
    """Host-side layout prep. Returns (shared_map, per_core_maps)."""
    bq1, bc1 = _gate_bias_quarter(b_ih1, b_hh1)

    shared = {
        "gi0": _host_gi0(target_tensor, emb, W_ih0, b_ih0, b_hh0),
        "h0q": _prep_hq(encoder_hidden[0]),
        "h1q": _prep_hq(encoder_hidden[1]),
        "h0T": _prep_hT_perm(encoder_hidden[0]),
        "h1T": _prep_hT_perm(encoder_hidden[1]),
        "whh0T": _prep_wT_perm(W_hh0),
        "wih1T": _prep_wT_perm(W_ih1),
        "whh1T": _prep_wT_perm(W_hh1),
        "bq1": bq1, "bc1": bc1,
        "bc0": _quarter_rows(np.asarray(b_hh0, np.float32)[2 * H:], J),
    }
    wout = np.asarray(W_out, np.float32)
    per_core = []
    for c in range(NCORES):
        sl = slice(c * VS, (c + 1) * VS)
        woutT = _perm_rows(np.ascontiguousarray(wout[sl].T))  # [128, 8, VS]
        per_core.append({
            "woutT": np.ascontiguousarray(_bf16(woutT)),
        })
    return shared, per_core


# ---------------------------------------------------------------------------
# Device program
# ---------------------------------------------------------------------------

def _emit(nc, tc, io, n_steps=T):
    import concourse.bass as bass
    from concourse import mybir
    from concourse.alu_op_type import AluOpType as alu

    f32 = mybir.dt.float32
    bf16 = mybir.dt.bfloat16
    Sig = mybir.ActivationFunctionType.Sigmoid
    Tanh = mybir.ActivationFunctionType.Tanh

    NRING = 4          # state ring slots

    ctx_pools = []

    def pool(name, bufs, space="SBUF"):
        p = tc.tile_pool(name=name, bufs=bufs, space=space)
        ctx_pools.append(p)
        return p.__enter__()

    consts = pool("consts", 1)
    state = pool("state", 1)
    hqp = pool("hq", 2)
    work = pool("work", 1)
    gp = pool("gi0", 3)

    # ---- constants / persistent tensors in SBUF ----
    bq1 = consts.tile([128, 3 * J], f32, tag="bq1", name="bq1")
    nc.sync.dma_start(bq1[:], io["bq1"][:])
    bc1 = consts.tile([128, J], f32, tag="bc1", name="bc1")
    nc.sync.dma_start(bc1[:], io["bc1"][:])
    bc0 = consts.tile([128, J], f32, tag="bc0", name="bc0")
    nc.sync.dma_start(bc0[:], io["bc0"][:])

    # state rings, transposed bf16, permuted-hidden: [128, slot, kchunk, 32]
    HT0 = state.tile([128, NRING, KH, 32], bf16, tag="H0T", name="H0T")
    nc.sync.dma_start(HT0[:, NRING - 1], io["h0T"][:])
    HT1 = state.tile([128, NRING, KH, 32], bf16, tag="H1T", name="H1T")
    nc.sync.dma_start(HT1[:, NRING - 1], io["h1T"][:])
    # full h1 history [128, k, 32t+b], mirrored from the ring by gpsimd
    hist = state.tile([128, KH, TB], bf16, tag="hist", name="hist")

    hq_init = {}
    for L in (0, 1):
        hq_init[L] = consts.tile([128, J], f32, tag=f"hq{L}i", name=f"hq{L}i")
        nc.sync.dma_start(hq_init[L][:], io[f"h{L}q"][:])

    def slot0(t):
        return HT0[:, t % NRING]

    def slot1(t):
        return HT1[:, t % NRING]

    # ---- gate matmul emitters (k-outer / q-inner for col-group overlap) ----

    def mms_rec0(G, Wa, kofs, lhsT_of):
        """L0 recurrent MMs into [128, 3J] psum; rz stays open for the
        gi0 identity pass."""
        for k in range(KH):
            lhsT = lhsT_of(k)
            w3 = Wa[:, kofs + k, :].rearrange("p (g j) -> p g j", g=3)
            for q in range(Q):
                Gq_rz = G[32 * q:32 * q + 32, 0:2 * J]
                nc.tensor.matmul(Gq_rz.rearrange("p (g j) -> p g j", g=2),
                                 lhsT, w3[:, 0:2, q * J:(q + 1) * J],
                                 start=(k == 0), stop=(k == KH - 1),
                                 tile_position=(0, 32 * q), skip_group_check=True)
            for q in range(Q):
                Gq_n = G[32 * q:32 * q + 32, 2 * J:3 * J]
                nc.tensor.matmul(Gq_n, lhsT, w3[:, 2, q * J:(q + 1) * J],
                                 start=(k == 0), stop=(k == KH - 1),
                                 tile_position=(0, 32 * q), skip_group_check=True)

    def mms_rec_first(G, Wa, kofs, lhsT_of):
        """L1 recurrent-path MMs (run BEFORE the input MMs)."""
        for k in range(KH):
            lhsT = lhsT_of(k)
            w3 = Wa[:, kofs + k, :].rearrange("p (g j) -> p g j", g=3)
            for q in range(Q):
                Gq_rz = G[32 * q:32 * q + 32, 0:2 * J]
                nc.tensor.matmul(Gq_rz.rearrange("p (g j) -> p g j", g=2),
                                 lhsT, w3[:, 0:2, q * J:(q + 1) * J],
                                 start=(k == 0), stop=False,
                                 tile_position=(0, 32 * q), skip_group_check=True)
            for q in range(Q):
                Gq_n = G[32 * q:32 * q + 32, 2 * J:3 * J]
                nc.tensor.matmul(Gq_n, lhsT, w3[:, 2, q * J:(q + 1) * J],
                                 start=(k == 0), stop=(k == KH - 1),
                                 tile_position=(0, 32 * q), skip_group_check=True)

    def mms_inp_last(G, Wa, kofs, lhsT_of):
        """L1 input-path MMs (run AFTER the recurrent MMs); rz stays open
        for the bias identity pass."""
        for k in range(KH):
            lhsT = lhsT_of(k)
            w3 = Wa[:, kofs + k, :].rearrange("p (g j) -> p g j", g=3)
            for q in range(Q):
                Gq_rz = G[32 * q:32 * q + 32, 0:2 * J]
                nc.tensor.matmul(Gq_rz.rearrange("p (g j) -> p g j", g=2),
                                 lhsT, w3[:, 0:2, q * J:(q + 1) * J],
                                 start=False, stop=(k == KH - 1),
                                 tile_position=(0, 32 * q), skip_group_check=True)
            for q in range(Q):
                Cq = G[32 * q:32 * q + 32, 3 * J:4 * J]
                nc.tensor.matmul(Cq, lhsT, w3[:, 2, q * J:(q + 1) * J],
                                 start=(k == 0), stop=(k == KH - 1),
                                 tile_position=(0, 32 * q), skip_group_check=True)

    def gate_elem0(G, gi0t, hq_prev):
        """L0: input path + biases come from the host gi0 tile."""
        Sp = work.tile([128, 2 * J], bf16, tag="Sp0")
        nc.vector.tensor_tensor(Sp[:], G[:, 0:2 * J], gi0t[:, 0:2 * J], alu.add)
        nc.scalar.activation(Sp[:], Sp[:], Sig)                     # r,z
        t0 = work.tile([128, J], bf16, tag="t00")
        nc.vector.tensor_tensor(t0[:], G[:, 2 * J:3 * J], bc0[:], alu.add)
        nc.vector.tensor_tensor(t0[:], t0[:], Sp[:, 0:J], alu.mult)   # r*gh_n
        nc.vector.tensor_tensor(t0[:], t0[:], gi0t[:, 2 * J:3 * J], alu.add)
        n_t = work.tile([128, J], bf16, tag="n0")
        nc.scalar.activation(n_t[:], t0[:], Tanh)
        d = work.tile([128, J], bf16, tag="d0")
        nc.vector.tensor_tensor(d[:], hq_prev[:], n_t[:], alu.subtract)
        nc.vector.tensor_tensor(d[:], d[:], Sp[:, J:2 * J], alu.mult)  # z*(h-n)
        hb = work.tile([128, J], bf16, tag="hb0")
        nc.vector.tensor_tensor(hb[:], n_t[:], d[:], alu.add)
        hq_new = hqp.tile([128, J], f32, tag="hq0")
        nc.scalar.copy(hq_new[:], hb[:])
        return hq_new, hb

    def gate_elem1(G, hq_prev):
        Sp = work.tile([128, 2 * J], bf16, tag="Sp1")
        nc.vector.tensor_tensor(Sp[:], G[:, 0:2 * J], bq1[:, 0:2 * J], alu.add)
        nc.scalar.activation(Sp[:], Sp[:], Sig)
        t0 = work.tile([128, J], bf16, tag="t01")
        nc.vector.tensor_tensor(t0[:], G[:, 2 * J:3 * J], bq1[:, 2 * J:3 * J],
                                alu.add)
        nc.vector.tensor_tensor(t0[:], t0[:], Sp[:, 0:J], alu.mult)   # r*gh_n
        t2 = work.tile([128, J], bf16, tag="t21")
        nc.vector.tensor_tensor(t2[:], G[:, 3 * J:4 * J], bc1[:], alu.add)
        nc.vector.tensor_tensor(t0[:], t0[:], t2[:], alu.add)
        n_t = work.tile([128, J], bf16, tag="n1")
        nc.scalar.activation(n_t[:], t0[:], Tanh)
        d = work.tile([128, J], bf16, tag="d1")
        nc.vector.tensor_tensor(d[:], hq_prev[:], n_t[:], alu.subtract)
        nc.vector.tensor_tensor(d[:], d[:], Sp[:, J:2 * J], alu.mult)
        hb = work.tile([128, J], bf16, tag="hb1")
        nc.vector.tensor_tensor(hb[:], n_t[:], d[:], alu.add)
        hq_new = hqp.tile([128, J], f32, tag="hq1")
        nc.scalar.copy(hq_new[:], hb[:])
        return hq_new, hb

    def transpose0(hb, t):
        dst = slot0(t)
        nc.vector.transpose(dst.rearrange("p k b -> p (k b)"), hb[:])

    def transpose1(hb, t):
        dst = slot1(t)
        nc.vector.transpose(dst.rearrange("p k b -> p (k b)"), hb[:])
        # mirror into the history buffer, off the critical path
        nc.gpsimd.tensor_copy(hist[:, :, 32 * t:32 * t + 32], dst)

    def load_gi0(t):
        g = gp.tile([128, 3 * J], bf16, tag="gi0")
        nc.sync.dma_start(g[:], io["gi0"][:, t, :])
        return g

    # =================== merged pipelined recurrence ===================
    with tc.tile_pool(name="arena", bufs=1) as arena_p, \
         tc.tile_pool(name="psumG0", bufs=1, space="PSUM") as psumG0, \
         tc.tile_pool(name="psumG1", bufs=2, space="PSUM") as psumG1, \
         tc.tile_pool(name="psumD", bufs=1, space="PSUM") as psumD:

        # weight arena: [128, 24, 3H] bf16, chunk-granular DMAs in first-use
        # order: whh0 (step-0 L0), then whh1 + wih1 (step-0 L1).
        WHH0, WHH1, WIH1 = 0, KH, 2 * KH
        a = arena_p.tile([128, 3 * KH, 3 * H], bf16, tag="arena", name="arena")
        gis = {0: load_gi0(0), 1: load_gi0(1)}
        for k in range(KH):
            nc.sync.dma_start(a[:, WHH0 + k, :], io["whh0T"][:, k, :])
        for k in range(KH):
            nc.sync.dma_start(a[:, WHH1 + k, :], io["whh1T"][:, k, :])
        for k in range(KH):
            nc.sync.dma_start(a[:, WIH1 + k, :], io["wih1T"][:, k, :])
        hq0_prev = hq_init[0]
        hq1_prev = hq_init[1]

        # HAM keep-warm: the PE clock gate drops to 4/8 across per-step
        # dependency gaps, which roughly doubles every matmul's duration.
        # A few filler matmuls on static operands into a never-read PSUM
        # bank keep the engine busy through the gap at 8/8.
        dummy = psumD.tile([128, 512], f32, tag="dummy", name="dummy")

        def keep_warm(n):
            for i in range(n):
                nc.tensor.matmul(dummy[:], a[:, 0, 0:128],
                                 a[:, 0, 1024:1536],
                                 start=True, stop=True, skip_group_check=True)

        for t in range(n_steps):
            # --- L0 step t ---
            G0 = psumG0.tile([128, 3 * J], f32, tag="G0", name="G0")
            mms_rec0(G0, a, WHH0, lambda k, tt=t: slot0(tt - 1)[:, k])
            hq0_prev, hb0 = gate_elem0(G0, gis[t], hq0_prev)
            transpose0(hb0, t)
            del gis[t]
            if t + 2 < n_steps:
                gis[t + 2] = load_gi0(t + 2)
            # --- L1 step t-1: recurrent, input, bias MMs ---
            if t >= 1:
                G1 = psumG1.tile([128, 4 * J], f32, tag="G1", name="G1")
                mms_rec_first(G1, a, WHH1,
                              lambda k, tt=t - 1: slot1(tt - 1)[:, k])
                mms_inp_last(G1, a, WIH1, lambda k, tt=t - 1: slot0(tt)[:, k])
                hq1_prev, hb1 = gate_elem1(G1, hq1_prev)
                transpose1(hb1, t - 1)
                keep_warm(6)

        # epilogue: L1 step n_steps-1
        G1 = psumG1.tile([128, 4 * J], f32, tag="G1", name="G1")
        mms_rec_first(G1, a, WHH1, lambda k: slot1(n_steps - 2)[:, k])
        mms_inp_last(G1, a, WIH1, lambda k: slot0(n_steps - 1)[:, k])
        hq1_prev, hb1 = gate_elem1(G1, hq1_prev)
        transpose1(hb1, n_steps - 1)
        keep_warm(64)

    # ================= logits GEMM (vocab-sharded) =================
    with tc.tile_pool(name="psumL", bufs=4, space="PSUM") as psumL, \
         tc.tile_pool(name="loadp", bufs=1) as loadp, \
         tc.tile_pool(name="outp", bufs=3) as outp:
        wt = []
        for s in range(VS // NS):
            w = loadp.tile([128, KH, NS], bf16, tag=f"ws{s}", name=f"ws{s}")
            nc.sync.dma_start(w[:], io["woutT"][:, :, s * NS:(s + 1) * NS])
            wt.append(w)
        for s in range(VS // NS):
            for m in range(NMT):
                Lt = psumL.tile([128, NS], f32, tag="L", name="L")
                for k in range(KH):
                    nc.tensor.matmul(
                        Lt[0:MT, :],
                        hist[:, k, m * MT:(m + 1) * MT],
                        wt[s][:, k, :],
                        start=(k == 0), stop=(k == KH - 1))
                ob = outp.tile([128, NS], f32, tag="ob", name="ob")
                nc.vector.tensor_copy(ob[0:MT, :], Lt[0:MT, :])
                nc.sync.dma_start(
                    io["logits"][m * MT:(m + 1) * MT, s * NS:(s + 1) * NS],
                    ob[0:MT, :])

    for p in reversed(ctx_pools):
        p.__exit__(None, None, None)


def _build_program(n_steps=T):
    import concourse.bacc as bacc
    import concourse.tile as tile
    from concourse import mybir

    f32 = mybir.dt.float32
    bf16 = mybir.dt.bfloat16

    nc = bacc.Bacc("TRN2", target_bir_lowering=False, debug=False,
                   num_devices=NCORES)

    def din(name, shape, dt):
        return nc.dram_tensor(name, list(shape), dt, kind="ExternalInput").ap()

    io = {
        "gi0": din("gi0", (128, T, 3 * J), bf16),
        "h0q": din("h0q", (128, J), f32),
        "h1q": din("h1q", (128, J), f32),
        "h0T": din("h0T", (128, KH, 32), bf16),
        "h1T": din("h1T", (128, KH, 32), bf16),
        "whh0T": din("whh0T", (128, KH, 3 * H), bf16),
        "wih1T": din("wih1T", (128, KH, 3 * H), bf16),
        "whh1T": din("whh1T", (128, KH, 3 * H), bf16),
        "bq1": din("bq1", (128, 3 * J), f32),
        "bc1": din("bc1", (128, J), f32),
        "bc0": din("bc0", (128, J), f32),
        "woutT": din("woutT", (128, KH, VS), bf16),
        "logits": nc.dram_tensor("logits", [TB, VS], f32,
                                 kind="ExternalOutput").ap(),
    }

    with tile.TileContext(nc) as tc:
        _emit(nc, tc, io, n_steps=n_steps)

    nc.compile()
    return nc


_CACHED = {}


def _get_program(n_steps=T):
    if n_steps not in _CACHED:
        _CACHED[n_steps] = _build_program(n_steps)
    return _CACHED[n_steps]


def kernel(encoder_outputs, encoder_hidden, target_tensor, emb,
           W_ih0, W_hh0, b_ih0, b_hh0, W_ih1, W_hh1, b_ih1, b_hh1,
           W_out, b_out, _trace=False):
    from concourse import bass_utils

    shared, per_core = _build_inputs(
        encoder_hidden, target_tensor, emb,
        W_ih0, W_hh0, b_ih0, b_hh0, W_ih1, W_hh1, b_ih1, b_hh1, W_out, b_out)

    nc = _get_program()
    in_maps = []
    for c in range(NCORES):
        m = dict(shared)
        m.update(per_core[c])
        in_maps.append(m)

    res = None
    for attempt in range(3):
        try:
            res = bass_utils.run_bass_kernel_spmd(
                nc, in_maps, core_ids=list(range(NCORES)), trace=_trace)
            break
        except Exception:
            if attempt == 2:
                raise
            import time
            time.sleep(20)

    parts = [res.results[c]["logits"].reshape(T, B, VS) for c in range(NCORES)]
    full = np.concatenate(parts, axis=2)          # [T, B, V]
    full += np.asarray(b_out, np.float32)[None, None, :]
    out = np.ascontiguousarray(full.transpose(1, 0, 2)).astype(np.float32)
    if _trace:
        kernel.last_results = res
    return out


kernel.last_results = None
